# revision 5
# baseline (speedup 1.0000x reference)
"""Trainium2 Bass kernel for grouped vector attention (sparse_attention).

Reference computation (B=2, L1=L2=512, D=256, g=16, n=16):
    Q = x_target @ Wq.T ; K = x_source @ Wk.T ; V = x_source @ Wv.T
    diff = Q.reshape(B,L1,1,n,g) - K.reshape(B,1,L2,n,g)
    scores = relu(einsum('bijng,g->bijn', relu(diff), w_mlp) + b_mlp)
    att = softmax(scores, axis=2)                      # over L2
    out = einsum('bijn,bjgn->bign', att, V.reshape(B,L2,g,n)).reshape(B,L1,D)

Sharding: 8 cores = 2 batches x 4 L2(j)-quarters. Each core handles all 512
queries against its 128 source positions and produces partial (unnormalized)
outputs + partial softmax denominators; the host sums the 4 partials per
batch and divides.

Per-core pipeline, 16 "octs" of 8 source positions each. Per oct one PSUM
tile [128, 512] holds all 8*16 scores with NO padding rows: slot
p = 16*jj + s  (jj = j within oct, s = group index n; p%16 == n).
  - t[d, i] = relu(Q[i,d] - K[j,d]) with d on partitions. VectorE units
    read the bf16 SBUF copy of Q (tensor_scalar add+max, ~345ns); ScalarE
    units read the f32 Q projection STRAIGHT FROM PSUM (activation w/
    bias; PSUM src is 570ns vs 720ns from SBUF).
  - 16 score matmuls per oct, 4 phases x 4 col-bands: phase q handles
    (j parity q&1, d-half q>>1) for all 4 bands with weight Wsel[q]
    ([128, 32], 8 nonzero cols placed so slots interleave perfectly).
    Cycling bands phase-major lets the PE stream up to 4 matmuls
    concurrently (32-col weights -> distinct col-groups). first_mm
    clears are region-scoped, so each band accumulates independently.
  - p = exp(scores + b) off PSUM; pc = max(p, 1) = exp(relu(scores+b))
  - V_sel[p, e] = V[8g + p//16, e] * (e%16 == p%16)  (one 16-way
    broadcast DMA from a DRAM copy of V + one masked multiply per oct)
  - out_partial[e, i] += V_sel[:, e-half].T @ pc  (PSUM accumulation
    across 16 octs); S_partial[n, i] += ones_sel.T @ pc

Inputs are host-packed partition-first ([128, n*X]) so each tensor is a
single DMA; issue is spread over the sync/scalar/gpsimd queues (DMA issue
serializes at ~600ns per descriptor on a queue).
"""

import numpy as np

import concourse.bass as bass
import concourse.bacc as bacc
import concourse.tile as tile
import concourse.mybir as mybir
from concourse.bass_utils import run_bass_kernel_spmd

import ml_dtypes

F32 = mybir.dt.float32
BF16 = mybir.dt.bfloat16
AL = mybir.AluOpType
AF = mybir.ActivationFunctionType

B, L1, L2, D = 2, 512, 512, 256
G = 16           # group size (d_group)
N = 16           # number of groups
NCORES = 8
JSH = 128        # source positions per core (L2 / 4)
NOCT = 16        # 16 octs of 8 source positions
BF = ml_dtypes.bfloat16

# per-oct engine assignment for the 16 t-units: 1 = ScalarE (570ns/op from
# PSUM), 0 = VectorE (345ns/op). ScalarE also does the oct's exp (570ns).
UNIT_ENGINE = (0, 1, 0, 0, 1, 0, 0, 1, 0, 1, 0, 0, 1, 0, 0, 1)


def _build(b_val: float):
    """Build + compile the per-core Bass graph. Same graph for all 8 cores."""
    nc = bacc.Bacc(
        "TRN2", target_bir_lowering=False, debug=False, enable_asserts=False
    )

    # ---- DRAM parameters (per-core shards, host-prepped, partition-first
    # single-DMA layouts: halves side by side along the free dim) ----
    xtT_d = nc.dram_tensor("xtT", [128, 2 * L1], BF16, kind="ExternalInput")
    xssT_d = nc.dram_tensor("xssT", [128, 2 * JSH], BF16, kind="ExternalInput")
    wqT_d = nc.dram_tensor("wqT", [128, 2 * D], BF16, kind="ExternalInput")
    wkT_d = nc.dram_tensor("wkT", [128, 2 * D], BF16, kind="ExternalInput")
    wvT_d = nc.dram_tensor("wvT", [128, 2 * D], BF16, kind="ExternalInput")
    sel_d = nc.dram_tensor("sel", [128, 128], BF16, kind="ExternalInput")
    vmask_d = nc.dram_tensor("vmask", [128, D], BF16, kind="ExternalInput")
    ones_d = nc.dram_tensor("ones_sel", [128, N], BF16, kind="ExternalInput")
    outp_d = nc.dram_tensor("outp", [2, 128, L1], BF16, kind="ExternalOutput")
    souts_d = nc.dram_tensor("souts", [N, L1], F32, kind="ExternalOutput")
    vdram = nc.dram_tensor("vdram", [JSH, D], BF16)

    with tile.TileContext(nc) as tc:
        with (
            tc.tile_pool(name="const", bufs=1) as cpool,
            tc.tile_pool(name="vselp", bufs=1) as vpool,
            tc.tile_pool(name="work", bufs=4) as wpool,
            tc.tile_pool(name="tmps", bufs=12) as tpool,
            tc.tile_pool(name="ps_s", bufs=2, space="PSUM") as ps_pool,
            tc.tile_pool(name="ps_q", bufs=1, space="PSUM") as pq_pool,
            tc.tile_pool(name="ps_k", bufs=1, space="PSUM") as pk_pool,
            tc.tile_pool(name="ps_acc", bufs=1, space="PSUM") as pa_pool,
        ):
            # ---- input DMAs, spread across issue queues; K/Q gate the
            # main loop so they go first on their queues ----
            xssT = cpool.tile([128, 2 * JSH], BF16, name="xssT")
            wkT = cpool.tile([128, 2 * D], BF16, name="wkT")
            xtT = cpool.tile([128, 2 * L1], BF16, name="xtT")
            wqT = cpool.tile([128, 2 * D], BF16, name="wqT")
            wvT = cpool.tile([128, 2 * D], BF16, name="wvT")
            selt = cpool.tile([128, 128], BF16, name="selt")
            vmask = cpool.tile([128, D], BF16, name="vmask")
            ones_sel = cpool.tile([128, N], BF16, name="ones_sel")
            bml = cpool.tile([128, 1], F32, name="bml")
            nc.vector.memset(bml[:], float(b_val))

            nc.sync.dma_start(xssT[:], xssT_d[:])
            nc.scalar.dma_start(wkT[:], wkT_d[:])
            nc.gpsimd.dma_start(xtT[:], xtT_d[:])
            nc.scalar.dma_start(wqT[:], wqT_d[:])
            nc.sync.dma_start(wvT[:], wvT_d[:])
            nc.gpsimd.dma_start(selt[:], sel_d[:])
            nc.sync.dma_start(vmask[:], vmask_d[:])
            nc.scalar.dma_start(ones_sel[:], ones_d[:])

            def hs(tl, h, w):  # h-half slice of a packed tile
                return tl[:, h * w : (h + 1) * w]

            # ---- accumulators ----
            ops = [
                pa_pool.tile([128, L1], F32, name=f"ops{eh}") for eh in range(2)
            ]
            sps = pa_pool.tile([16, L1], F32, name="sps")

            # ---- projections: K first (gates t-ops), then Q, then V ----
            # QT[h] (128 e, 512 i) bf16 for VectorE; psq[h] f32 PSUM stays
            # resident for ScalarE units. KTn[h] (128 e, 128 j) f32 negated.
            QT = [cpool.tile([128, L1], BF16, name=f"QT{h}") for h in range(2)]
            KTn = [cpool.tile([128, JSH], F32, name=f"KTn{h}") for h in range(2)]
            for eh in range(2):
                psk = pk_pool.tile([128, JSH], F32, name="psk", tag="psk")
                for dh in range(2):
                    nc.tensor.matmul(
                        psk[:],
                        hs(wkT, dh, D)[:, eh * 128 : (eh + 1) * 128],
                        hs(xssT, dh, JSH),
                        start=(dh == 0),
                        stop=(dh == 1),
                    )
                nc.scalar.mul(KTn[eh][:], psk[:], -1.0)
            psq = [
                pq_pool.tile([128, L1], F32, name=f"psq{h}") for h in range(2)
            ]
            for eh in range(2):
                for dh in range(2):
                    nc.tensor.matmul(
                        psq[eh][:],
                        hs(wqT, dh, D)[:, eh * 128 : (eh + 1) * 128],
                        hs(xtT, dh, L1),
                        start=(dh == 0),
                        stop=(dh == 1),
                    )
            # QT0 on VectorE (idle at head; unblocks its first t-op fast)
            nc.vector.tensor_copy(QT[0][:], psq[0][:])
            nc.scalar.copy(QT[1][:], psq[1][:])

            # ---- V projection -> DRAM -> per-oct broadcast+mask ----
            Vt = cpool.tile([128, D], BF16, name="Vt")
            psv = pk_pool.tile([128, D], F32, name="psv", tag="psk")
            for dh in range(2):
                nc.tensor.matmul(
                    psv[:],
                    hs(xssT, dh, JSH),
                    hs(wvT, dh, D),
                    start=(dh == 0),
                    stop=(dh == 1),
                )
            nc.scalar.copy(Vt[:], psv[:])
            nc.gpsimd.dma_start(vdram[:], Vt[:])

            V_sel = [
                vpool.tile([128, D], BF16, name=f"vs{g}") for g in range(NOCT)
            ]

            def build_vsel(g):
                vs = V_sel[g]
                bsrc = (
                    vdram.ap()[8 * g : 8 * g + 8, :]
                    .unsqueeze(1)
                    .broadcast_to((8, 16, D))
                )
                if g % 2 == 0:
                    nc.sync.dma_start(vs[:], bsrc)
                else:
                    nc.gpsimd.dma_start(vs[:], bsrc)
                nc.vector.tensor_tensor(vs[:], vs[:], vmask[:], op=AL.mult)

            for g in range(3):
                build_vsel(g)

            # ---- main loop: 16 octs of 8 source positions ----
            for g in range(NOCT):
                if g + 3 < NOCT:
                    build_vsel(g + 3)
                ps = ps_pool.tile([128, L1], F32, name="ps", tag="ps_s")
                for q in range(4):
                    for b in range(4):
                        u = 4 * q + b
                        j = 8 * g + 2 * b + (q & 1)
                        h = q >> 1
                        t = tpool.tile([128, L1], BF16, name="t", tag="t")
                        if UNIT_ENGINE[u]:
                            # t = relu(Q + (-K)), Q straight from PSUM
                            nc.scalar.activation(
                                t[:],
                                psq[h][:],
                                AF.Relu,
                                bias=KTn[h][:, j : j + 1],
                                scale=1.0,
                            )
                        else:
                            # t = max(Q + (-K), 0)
                            nc.vector.tensor_scalar(
                                t[:],
                                QT[h][:],
                                KTn[h][:, j : j + 1],
                                0.0,
                                AL.add,
                                AL.max,
                            )
                        # first_mm clears are region-scoped: each band is
                        # its own accumulation group over the 4 phases
                        nc.tensor.matmul(
                            ps[32 * b : 32 * b + 32, :],
                            selt[:, 32 * q : 32 * q + 32],
                            t[:],
                            start=(q == 0),
                            stop=(q == 3),
                            tile_position=(0, 32 * b),
                            skip_group_check=True,
                        )
                # p = exp(scores + b); pc = max(p, 1) = exp(relu(scores + b))
                p = wpool.tile([128, L1], BF16, name="p", tag="p", bufs=4)
                nc.scalar.activation(p[:], ps[:], AF.Exp, bias=bml[:], scale=1.0)
                pc = wpool.tile([128, L1], BF16, name="pc", tag="pc", bufs=4)
                nc.vector.tensor_scalar(pc[:], p[:], 1.0, None, AL.max)
                # accumulate partial outputs and denominators
                for eh in range(2):
                    nc.tensor.matmul(
                        ops[eh][:],
                        V_sel[g][:, eh * 128 : (eh + 1) * 128],
                        pc[:],
                        start=(g == 0),
                        stop=(g == NOCT - 1),
                        skip_group_check=True,
                    )
                nc.tensor.matmul(
                    sps[:],
                    ones_sel[:, 0:N],
                    pc[:],
                    start=(g == 0),
                    stop=(g == NOCT - 1),
                    skip_group_check=True,
                )

            # ---- evacuate + store (bf16 partials; DMAs on separate
            # queues so the tail issue doesn't serialize) ----
            ou0 = wpool.tile([128, L1], BF16, name="ou0")
            nc.vector.tensor_copy(ou0[:], ops[0][:])
            nc.sync.dma_start(outp_d[0], ou0[:])
            ou1 = wpool.tile([128, L1], BF16, name="ou1")
            nc.scalar.copy(ou1[:], ops[1][:])
            nc.scalar.dma_start(outp_d[1], ou1[:])
            so = wpool.tile([16, L1], F32, name="so")
            nc.scalar.copy(so[:], sps[:])
            nc.gpsimd.dma_start(souts_d[:], so[:])

    nc.compile()
    return nc


_CACHE: dict = {}


def _get_graph(b_val: float):
    key = round(float(b_val), 10)
    if key not in _CACHE:
        _CACHE[key] = _build(float(b_val))
    return _CACHE[key]


def _host_prep(x_source, x_target, Wq, Wk, Wv, w_mlp):
    """Build per-core input maps (numpy, bf16, partition-first packing)."""
    w_full = np.tile(np.asarray(w_mlp, np.float32), D // G)  # w_full[d] = w[d%16]
    # 4 score-weight matrices packed side by side: phase q handles
    # (j parity q&1, d-half q>>1);
    # sel[dl, 32q + c] = w_full[128*(q>>1) + dl] at c = 16*(q&1) + 8*(q>>1) + dl//16
    sel = np.zeros((4, 128, 32), np.float32)
    for q in range(4):
        h = q >> 1
        for dl in range(128):
            c = 16 * (q & 1) + 8 * h + dl // G
            sel[q, dl, c] = w_full[128 * h + dl]
    sel_p = np.ascontiguousarray(sel.transpose(1, 0, 2).reshape(128, 128))
    # V_sel mask: slot p = 16*jj + s -> keep col e iff e%16 == p%16
    vmask = np.zeros((128, D), np.float32)
    for p in range(128):
        vmask[p, (p % G) :: G] = 1.0
    # S selector: slot p -> column p%16
    ones_sel = np.zeros((128, N), np.float32)
    for p in range(128):
        ones_sel[p, p % N] = 1.0

    def pack_h(a):  # (256, X) -> (128, 2X): halves side by side
        X = a.shape[1]
        return np.ascontiguousarray(
            a.reshape(2, 128, X).transpose(1, 0, 2).reshape(128, 2 * X)
        )

    wq_b = pack_h(np.asarray(Wq, np.float32).T).astype(BF)
    wk_b = pack_h(np.asarray(Wk, np.float32).T).astype(BF)
    wv_b = pack_h(np.asarray(Wv, np.float32).T).astype(BF)
    sel_b = sel_p.astype(BF)
    vmask_b = vmask.astype(BF)
    ones_b = ones_sel.astype(BF)

    xtT = [
        pack_h(np.asarray(x_target[b], np.float32).T).astype(BF)
        for b in range(B)
    ]
    xsT = [np.asarray(x_source[b], np.float32).T for b in range(B)]
    in_maps = []
    for core in range(NCORES):
        b, jq = divmod(core, 4)
        j0 = jq * JSH
        xssT = pack_h(xsT[b][:, j0 : j0 + JSH]).astype(BF)
        in_maps.append(
            {
                "xtT": xtT[b],
                "xssT": xssT,
                "wqT": wq_b,
                "wkT": wk_b,
                "wvT": wv_b,
                "sel": sel_b,
                "vmask": vmask_b,
                "ones_sel": ones_b,
            }
        )
    return in_maps


def _host_gather(results):
    """Sum partials over j-shards, normalize, reshape to (B, L1, D)."""
    out = np.empty((B, L1, D), np.float32)
    for b in range(B):
        cores = [b * 4 + jq for jq in range(4)]
        U = sum(
            np.asarray(results[c]["outp"], np.float64).reshape(D, L1)
            for c in cores
        )  # (e, i)
        S = sum(
            np.asarray(results[c]["souts"], np.float64) for c in cores
        )  # (n, i)
        att = U / S[np.arange(D) % N, :]  # (e, i)
        out[b] = att.T.astype(np.float32)
    return out


def run(inputs, trace=False, **kwargs):
    nc = _get_graph(float(np.asarray(inputs["b_mlp"]).reshape(-1)[0]))
    in_maps = _host_prep(
        inputs["x_source"],
        inputs["x_target"],
        inputs["Wq"],
        inputs["Wk"],
        inputs["Wv"],
        inputs["w_mlp"],
    )
    res = run_bass_kernel_spmd(
        nc, in_maps, core_ids=list(range(NCORES)), trace=trace, **kwargs
    )
    return _host_gather(res.results), res


def kernel(**inputs) -> np.ndarray:
    out, _ = run(inputs, trace=False)
    return out
